# revision 1
# baseline (speedup 1.0000x reference)
"""Al-Salam-Carlitz KAN layer on 8 TRN2 NeuronCores.

Math: y[b,o] = sum_{i,d} P_d(tanh(x[b,i])) * coeffs[i,o,d], where P_d are the
Al-Salam-Carlitz polynomials given by a three-term recurrence in scalars a, q.
Each P_d is a degree-d polynomial in t = tanh(x), so on the host we fold the
(D+1)x(D+1) basis-change matrix into coeffs:

    y[b,o] = bias[o] + sum_{k=1..D} sum_i t[b,i]^k * Cf[i,o,k]

with bias[o] = sum_i Cf[i,o,0] (the k=0 column times t^0 == 1).  This removes
1/8 of the matmul work and leaves the device with: tanh, a bf16 power chain,
and a K=7*1024 contraction done as 448 TensorE matmuls per core.

Sharding: data-parallel over batch (4096 -> 8 x 512).  Each core receives its
x-shard pre-transposed ([I, 512], so the contraction dim lands on SBUF
partitions), the folded weights (bf16, pre-laid-out in exact consumption
order for contiguous chunked DMA), and the bias.  No collectives; the host
concatenates the 8 output shards.

Matmul schedule (one core): 8 output tiles yT[oc] = [128 o, 512 b], each
accumulating 56 K-steps in PSUM bank oc.
  Phase A (j = 0..13): for each j, one matmul into every bank -- consumption
    of power planes is 8x slower than production, so the PE never stalls on
    the tanh/power chain during ramp-up.
  Phase B (oc = 0..7): finish each bank's remaining 42 K-steps back-to-back,
    so banks complete staggered and PSUM evacuation + output DMA overlap the
    next bank's matmuls.
"""

import numpy as np
import ml_dtypes

B, I, O, D1 = 4096, 1024, 1024, 8
NCORES = 8
BS = B // NCORES       # batch rows per core (moving free dim of each matmul)
IC = I // 128          # i chunks (contraction tiles per power plane)
OC = O // 128          # o chunks (output partition tiles)
NK = D1 - 1            # power planes k = 1..7
NJ = IC * NK           # K-steps per output tile
NJA = 14               # phase-A K-steps (covers planes of i-chunks 0..1)

# (oc, j) consumption order of the 448 stationary weight tiles
SEQ = [(oc, j) for j in range(NJA) for oc in range(OC)] + \
      [(oc, j) for oc in range(OC) for j in range(NJA, NJ)]
# weight-DMA chunk sizes (tiles): phase A starts fine-grained (the first
# chunk gates the first matmul) then coarsens; phase B uses 3 chunks of 14
# per group.  Fewer chunks = fewer PE semaphore waits + fewer descriptor
# pushes on the sync sequencer.
_SIZES = [OC // 2, OC // 2, OC] + [2 * OC] * ((NJA - 2) // 2) + \
         [2 * NK] * (OC * (NJ - NJA) // (2 * NK))
CHUNKS = []
_s = 0
for _sz in _SIZES:
    CHUNKS.append((_s, _sz))
    _s += _sz
assert _s == OC * NJ

# chunk index whose last matmul completes group oc (phase B: 3 chunks/group)
_NA = 3 + (NJA - 2) // 2                     # number of phase-A chunks
GROUP_END_CHUNK = [_NA + 3 * oc + 2 for oc in range(OC)]

_GRAPH = None
LAST_RESULT = None     # BassKernelResults of the most recent run (for test.py)

# weight-chunk SBUF ring slots: deep enough that the sync sequencer's
# per-chunk descriptor generation (0.6-3.3us each, run-to-run variable)
# starts early enough for phase-B chunks to land before the PE reaches
# them (4-deep measured a 1.1us stall; 6-deep still stalled ~2us on some
# runs). 8 slots = 32KB/partition of SBUF, well within budget.
CW_BUFS = 8


def _build_graph_raw():
    """Raw bacc build: manual per-engine streams + semaphores.  Saves the
    Tile exit drain + double all-engine barrier (~9us) and waits only once
    per weight chunk on the PE instead of per matmul."""
    import concourse.bass as bass
    from concourse import bacc, mybir

    nc = bacc.Bacc("TRN2", target_bir_lowering=False, debug=False,
                   num_devices=NCORES, monotonic_sem_count=0)
    f32 = mybir.dt.float32
    bf16 = mybir.dt.bfloat16

    xT = nc.dram_tensor("xT", [I, BS], f32, kind="ExternalInput").ap()
    cw = nc.dram_tensor("cw", [128, OC * NJ * 128], bf16,
                        kind="ExternalInput").ap()
    bias = nc.dram_tensor("bias", [128, OC], f32, kind="ExternalInput").ap()
    yT = nc.dram_tensor("yT", [O, BS], f32, kind="ExternalOutput").ap()

    max_chunk = max(sz for _, sz in CHUNKS)
    xin = [nc.alloc_sbuf_tensor(f"xin{i}", [128, BS], f32).ap()
           for i in range(IC)]
    planes = [nc.alloc_sbuf_tensor(f"pl{j}", [128, BS], bf16).ap()
              for j in range(NJ)]
    cwbuf = [nc.alloc_sbuf_tensor(f"cwb{i}", [128, max_chunk * 128],
                                  bf16).ap()
             for i in range(CW_BUFS)]
    bias_t = nc.alloc_sbuf_tensor("biasb", [128, OC], f32).ap()
    ot = [nc.alloc_sbuf_tensor(f"ot{i}", [128, BS], f32).ap()
          for i in range(2)]
    ps = [nc.alloc_psum_tensor(f"ps{i}", [128, BS], f32).ap()
          for i in range(OC)]

    from contextlib import ExitStack
    with ExitStack() as stack:
        # gpsimd only issues the early bias DMA (completion consumed mid-
        # kernel), so its expensive end-of-block dge_drain can be skipped
        block = stack.enter_context(nc.Block(no_gpsimd_drain=True))
        # DMA completion increments land as 16 per-slice +1s, and slices of
        # different in-flight DMAs interleave -- so a semaphore may only be
        # waited at "all DMAs issued on it so far" thresholds.  The weight
        # stream round-robins CW_BUFS semaphores (slot ring ensures only one
        # in-flight DMA per sem); x tiles get one sem each; output slots two.
        # NEFF teardown emits ~2 clear ops per allocated semaphore (~210ns
        # each, inside the measured exec window) -- keep the set minimal.
        cw_dma = [stack.enter_context(nc.semaphore(f"cw_dma{r}"))
                  for r in range(CW_BUFS)]
        # xin0/xin1 gate phase-A tanh planes and get their own sems; xins
        # 2..7 are only needed for phase B (~36us in) and share an all-done
        # sem (bias can't share: SWDGE and HWDGE DMAs may not mix on a sem)
        xin0_dma = stack.enter_context(nc.semaphore("xin0_dma"))
        xin1_dma = stack.enter_context(nc.semaphore("xin1_dma"))
        xr_dma = stack.enter_context(nc.semaphore("xr_dma"))
        bias_dma = stack.enter_context(nc.semaphore("bias_dma"))
        out_dma = [stack.enter_context(nc.semaphore(f"out_dma{r}"))
                   for r in range(2)]
        act_pl = stack.enter_context(nc.semaphore("act_pl"))
        dve_pl = stack.enter_context(nc.semaphore("dve_pl"))
        pe_ch = stack.enter_context(nc.semaphore("pe_ch"))
        act_ev = stack.enter_context(nc.semaphore("act_ev"))

        @block.sync
        def _(eng: bass.BassEngine):
            for ci, (s0, size) in enumerate(CHUNKS):
                if ci == 0:
                    # only xin0 rides the weight ring (each transfer here
                    # delays the next chunk ~0.7us and stalls the PE ramp;
                    # xins 1..7 go via the ACT ring)
                    eng.dma_start(out=xin[0][:], in_=xT[0:128, :]
                                  ).then_inc(xin0_dma, 16)
                if ci >= CW_BUFS:
                    eng.wait_ge(pe_ch, ci - CW_BUFS + 1)
                eng.dma_start(
                    out=cwbuf[ci % CW_BUFS][:, :size * 128],
                    in_=cw[:, s0 * 128:(s0 + size) * 128],
                ).then_inc(cw_dma[ci % CW_BUFS], 16)

        @block.gpsimd
        def _(eng: bass.BassEngine):
            # bias is 128 tiny descriptors; on the ACT ring it would delay
            # xin0 (FIFO).  gpsimd SWDGE is slow but bias has ~40us of slack.
            eng.dma_start(out=bias_t[:], in_=bias[:]).then_inc(bias_dma, 16)

        @block.scalar
        def _(eng: bass.BassEngine):
            eng.wait_ge(xin0_dma, 16)
            eng.activation(planes[0][:], xin[0][:],
                           mybir.ActivationFunctionType.Tanh
                           ).then_inc(act_pl, 1)
            # xin1 from ACT's ring right after tanh0; tanh1's plane is first
            # consumed ~10us later (phase A j=7)
            eng.dma_start(out=xin[1][:], in_=xT[128:256, :]
                          ).then_inc(xin1_dma, 16)
            eng.wait_ge(xin1_dma, 16)
            eng.activation(planes[NK][:], xin[1][:],
                           mybir.ActivationFunctionType.Tanh
                           ).then_inc(act_pl, 1)
            # xins 2..7 on ACT's own HWDGE ring, issued after the hot tanhs;
            # their planes are first needed by phase B at ~35us
            for i in range(2, IC):
                eng.dma_start(
                    out=xin[i][:], in_=xT[i * 128:(i + 1) * 128, :]
                ).then_inc(xr_dma, 16)
            eng.wait_ge(xr_dma, 16 * (IC - 2))
            for i in range(2, IC):
                eng.activation(planes[i * NK][:], xin[i][:],
                               mybir.ActivationFunctionType.Tanh
                               ).then_inc(act_pl, 1)
            eng.wait_ge(bias_dma, 16)
            ev = 0
            for oc in range(OC):
                eng.wait_ge(pe_ch, GROUP_END_CHUNK[oc] + 1)
                if oc >= 2:
                    eng.wait_ge(out_dma[oc % 2], 16 * (oc // 2))
                # last group is the serial tail: pipeline it in two column
                # halves so the first half's store overlaps the second evac
                halves = ([(0, BS)] if oc < OC - 1
                          else [(0, BS // 2), (BS // 2, BS)])
                for c0, c1 in halves:
                    eng.activation(ot[oc % 2][:, c0:c1], ps[oc][:, c0:c1],
                                   mybir.ActivationFunctionType.Identity,
                                   bias=bias_t[:, oc:oc + 1]
                                   ).then_inc(act_ev, 1)
                    ev += 1
                    eng.wait_ge(act_ev, ev)
                    eng.dma_start(
                        out=yT[oc * 128:(oc + 1) * 128, c0:c1],
                        in_=ot[oc % 2][:, c0:c1]
                    ).then_inc(out_dma[oc % 2], 16)
            eng.wait_ge(out_dma[0], 16 * (OC // 2))
            eng.wait_ge(out_dma[1], 16 * (OC // 2 + 1))

        @block.vector
        def _(eng: bass.BassEngine):
            for i in range(IC):
                eng.wait_ge(act_pl, i + 1)
                for k1 in range(1, NK):
                    if k1 >= 2:
                        # same-engine RAW still needs a sem wait (deep
                        # pipeline, no interlock)
                        eng.wait_ge(dve_pl, i * (NK - 1) + k1 - 1)
                    eng.tensor_mul(planes[i * NK + k1][:],
                                   planes[i * NK + k1 - 1][:],
                                   planes[i * NK][:]
                                   ).then_inc(dve_pl, 1)

        @block.tensor
        def _(eng: bass.BassEngine):
            done = [0] * OC
            seen_act = seen_dve = 0
            for ci, (s0, size) in enumerate(CHUNKS):
                # attach all of the chunk's waits to its first matmul --
                # the move_matmul_waits_to_ldweights compile pass hoists
                # them onto the LDWEIGHTS, keeping the PE's 64-deep
                # reorder window free to pull later weight loads ahead
                # (a standalone EventSemaphore wait would block it)
                js = [SEQ[s][1] for s in range(s0, s0 + size)]
                need_act = max((j // NK + 1 for j in js if j % NK == 0),
                               default=0)
                need_dve = max((j // NK * (NK - 1) + j % NK
                                for j in js if j % NK != 0), default=0)
                if need_act > seen_act:
                    eng.wait_ge(act_pl, need_act)
                    seen_act = need_act
                if need_dve > seen_dve:
                    eng.wait_ge(dve_pl, need_dve)
                    seen_dve = need_dve
                for t in range(size):
                    oc, j = SEQ[s0 + t]
                    mm = eng.matmul(ps[oc][:],
                                    cwbuf[ci % CW_BUFS][:,
                                                        t * 128:(t + 1) * 128],
                                    planes[j][:],
                                    start=(done[oc] == 0),
                                    stop=(done[oc] == NJ - 1))
                    if t == 0:
                        mm._wait_ge(cw_dma[ci % CW_BUFS],
                                    16 * (ci // CW_BUFS + 1))
                    done[oc] += 1
                    if t == size - 1:
                        mm.then_inc(pe_ch, 1)

    nc.compile()
    return nc


def _build_graph():
    import concourse.tile as tile
    from concourse import bacc, mybir

    nc = bacc.Bacc("TRN2", target_bir_lowering=False, debug=False,
                   num_devices=NCORES)
    f32 = mybir.dt.float32
    bf16 = mybir.dt.bfloat16

    xT = nc.dram_tensor("xT", [I, BS], f32, kind="ExternalInput").ap()
    cw = nc.dram_tensor("cw", [128, OC * NJ * 128], bf16,
                        kind="ExternalInput").ap()
    bias = nc.dram_tensor("bias", [128, OC], f32, kind="ExternalInput").ap()
    yT = nc.dram_tensor("yT", [O, BS], f32, kind="ExternalOutput").ap()

    with tile.TileContext(nc) as tc:
        with tc.tile_pool(name="xin", bufs=IC) as xin_pool, \
             tc.tile_pool(name="planes", bufs=NJ) as plane_pool, \
             tc.tile_pool(name="cwp", bufs=8) as cw_pool, \
             tc.tile_pool(name="misc", bufs=1) as misc_pool, \
             tc.tile_pool(name="psum", bufs=OC, space="PSUM") as psum_pool, \
             tc.tile_pool(name="osb", bufs=2) as out_pool:

            bias_t = misc_pool.tile([128, OC], f32, tag="bias")
            nc.gpsimd.dma_start(bias_t[:], bias[:])

            # power planes t^k, k=1..7, per i-chunk; all stay resident.
            # DMA emission order (= sync-engine issue order): xin0, then the
            # first weight chunks interleaved with the remaining xins, then
            # the rest of the weight chunks — matches consumption order.
            planes = []
            cw_tiles = []

            def emit_cw_chunk(ci):
                s0, size = CHUNKS[ci]
                cwt = cw_pool.tile([128, size * 128], bf16, tag="cw",
                                   name="cwt")
                nc.sync.dma_start(cwt[:], cw[:, s0 * 128:(s0 + size) * 128])
                cw_tiles.append(cwt)

            for ic in range(IC):
                # x-shard loads issue from the Scalar engine so the Sync
                # queue carries only the weight stream (cw chunk 0 lands
                # first) and xin_ic never queues behind megabytes of weights
                xin = xin_pool.tile([128, BS], f32, tag="xin", name="xin")
                nc.sync.dma_start(xin[:], xT[ic * 128:(ic + 1) * 128, :])
                xt = plane_pool.tile([128, BS], bf16, tag="planes", name="xt")
                nc.scalar.activation(xt[:], xin[:],
                                     mybir.ActivationFunctionType.Tanh)
                planes.append(xt)
                prev = xt
                for k in range(2, D1):
                    pw = plane_pool.tile([128, BS], bf16, tag="planes",
                                         name="pw")
                    nc.vector.tensor_mul(pw[:], prev[:], xt[:])
                    planes.append(pw)
                    prev = pw
                emit_cw_chunk(ic)  # first 8 weight chunks ride along

            ps_tiles = [psum_pool.tile([128, BS], f32, tag="ps", name="ps")
                        for _ in range(OC)]
            done = [0] * OC
            s = 0
            for ci, (s0, size) in enumerate(CHUNKS):
                if ci >= IC:
                    emit_cw_chunk(ci)
                cwt = cw_tiles[ci]
                for t in range(size):
                    oc, j = SEQ[s0 + t]
                    nc.tensor.matmul(ps_tiles[oc][:],
                                     cwt[:, t * 128:(t + 1) * 128],
                                     planes[j][:],
                                     start=(done[oc] == 0),
                                     stop=(done[oc] == NJ - 1))
                    done[oc] += 1
                    if done[oc] == NJ:
                        ot = out_pool.tile([128, BS], f32, tag="ot",
                                           name="ot")
                        nc.scalar.activation(
                            ot[:], ps_tiles[oc][:],
                            mybir.ActivationFunctionType.Identity,
                            bias=bias_t[:, oc:oc + 1])
                        nc.gpsimd.dma_start(
                            yT[oc * 128:(oc + 1) * 128, :], ot[:])
                    s += 1
            assert s == OC * NJ and all(d == NJ for d in done)

    nc.compile()
    return nc


def _get_graph():
    global _GRAPH
    if _GRAPH is None:
        import os
        if os.environ.get("KERNEL_IMPL") == "tile":
            _GRAPH = _build_graph()
        else:
            _GRAPH = _build_graph_raw()
    return _GRAPH


def _host_prep(a, q, coeffs):
    """Fold the polynomial basis change into the weights (float64 on host)."""
    # c[d, k]: P_d(t) = sum_k c[d, k] * t^k, from the three-term recurrence
    c = np.zeros((D1, D1), np.float64)
    c[0, 0] = 1.0
    if D1 > 1:
        c[1, 1] = 1.0
        c[1, 0] = -a
    for n in range(2, D1):
        c[n, 1:] += c[n - 1, :-1]
        c[n, :] -= (a + q ** n) * c[n - 1, :]
        c[n, :] -= a * q ** (n - 1) * c[n - 2, :]

    Cf = (coeffs.reshape(-1, D1).astype(np.float64) @ c).reshape(I, O, D1)
    bias = Cf[:, :, 0].sum(axis=0).astype(np.float32)                # [O]
    Ck = Cf[:, :, 1:].astype(np.float32).astype(ml_dtypes.bfloat16)  # [I,O,NK]

    # stationary tile for (oc, j=ic*NK+k1): [128 i-part, 128 o-col] slice
    t = Ck.reshape(IC, 128, OC, 128, NK)            # [ic, p, oc, ol, k1]
    X = np.ascontiguousarray(t.transpose(2, 0, 4, 1, 3)) \
          .reshape(OC, NJ, 128, 128)                # [oc, j, p, ol]
    oc_idx = np.array([oc for oc, _ in SEQ])
    j_idx = np.array([j for _, j in SEQ])
    seq_tiles = X[oc_idx, j_idx]                    # [448, p, ol]
    cw_dev = np.ascontiguousarray(
        seq_tiles.transpose(1, 0, 2)).reshape(128, OC * NJ * 128)
    bias_dev = np.ascontiguousarray(bias.reshape(OC, 128).T)  # [128, OC]
    return cw_dev, bias_dev


def _ensure_axon_hooks_importable():
    """run_bass_kernel_spmd imports antenv.axon_hooks when BASS_TRACE is
    set; some images lack that module.  Register a no-op fallback so a
    trace request degrades to a warning instead of an ImportError."""
    import sys
    import types
    if "antenv.axon_hooks" in sys.modules:
        return
    try:
        import antenv.axon_hooks  # noqa: F401
    except ImportError:
        mod = types.ModuleType("antenv.axon_hooks")
        state = {"hook": None}
        mod.set_axon_ntff_profile_hook = \
            lambda h: state.__setitem__("hook", h)
        mod.get_axon_ntff_profile_hook = lambda: state["hook"]
        sys.modules["antenv.axon_hooks"] = mod
        try:
            import antenv
            antenv.axon_hooks = mod
        except ImportError:
            pass


def kernel(x, a, q, coeffs):
    global LAST_RESULT
    _ensure_axon_hooks_importable()
    from concourse.bass_utils import run_bass_kernel_spmd

    x = np.ascontiguousarray(np.asarray(x, dtype=np.float32))
    coeffs = np.ascontiguousarray(np.asarray(coeffs, dtype=np.float32))
    a_val = float(np.asarray(a).reshape(-1)[0])
    q_val = float(np.asarray(q).reshape(-1)[0])

    cw_dev, bias_dev = _host_prep(a_val, q_val, coeffs)
    xs = x.reshape(NCORES, BS, I).transpose(0, 2, 1)  # [core, I, BS]

    in_maps = [{
        "xT": np.ascontiguousarray(xs[c]),
        "cw": cw_dev,
        "bias": bias_dev,
    } for c in range(NCORES)]

    nc = _get_graph()
    res = run_bass_kernel_spmd(nc, in_maps, core_ids=list(range(NCORES)))
    LAST_RESULT = res

    shards = [np.asarray(res.results[c]["yT"]).T for c in range(NCORES)]
    return np.ascontiguousarray(np.concatenate(shards, axis=0),
                                dtype=np.float32)


if __name__ == "__main__":
    rng = np.random.default_rng(0)
    inputs = {
        "x": rng.standard_normal((B, I), dtype=np.float32),
        "a": np.zeros((1,), np.float32),
        "q": np.ones((1,), np.float32),
        "coeffs": rng.standard_normal((I, O, D1), dtype=np.float32)
        / (I * D1),
    }
    y = kernel(**inputs)
    print("out", y.shape, y.dtype, float(np.abs(y).mean()))



# revision 9
# speedup vs baseline: 1.0696x; 1.0696x over previous
"""Al-Salam-Carlitz KAN layer on 8 TRN2 NeuronCores.

Math: y[b,o] = sum_{i,d} P_d(tanh(x[b,i])) * coeffs[i,o,d], where P_d are the
Al-Salam-Carlitz polynomials given by a three-term recurrence in scalars a, q.
Each P_d is a degree-d polynomial in t = tanh(x), so on the host we fold the
(D+1)x(D+1) basis-change matrix into coeffs:

    y[b,o] = bias[o] + sum_{k=1..D} sum_i t[b,i]^k * Cf[i,o,k]

with bias[o] = sum_i Cf[i,o,0] (the k=0 column times t^0 == 1).  This removes
1/8 of the matmul work and leaves the device with: tanh, a bf16 power chain,
and a K=7*1024 contraction done as 448 TensorE matmuls per core.

Sharding: data-parallel over batch (4096 -> 8 x 512).  Each core receives its
x-shard pre-transposed ([I, 512] in bf16, so the contraction dim lands on SBUF
partitions), the folded weights (bf16, pre-laid-out in exact consumption
order for contiguous chunked DMA), and the bias.  No collectives; the host
concatenates the 8 output shards.

Matmul schedule (one core): 8 output tiles yT[oc] = [128 o, 512 b], each
accumulating 56 K-steps in PSUM bank oc.
  Warm-up: ~16 small dummy matmuls on garbage SBUF ramp the PE out of its
    low-power p-state (first ~3-6us run at 1.2 instead of 2.4 GHz) while the
    first x-tile DMA + tanh are still in flight.
  Phase A (j = 0..13): for each j, one matmul into every bank -- consumption
    of power planes is 8x slower than production, so the PE never stalls on
    the tanh/power chain during ramp-up.
  Phase B (oc = 0..7): finish each bank's remaining 42 K-steps back-to-back,
    so banks complete staggered and PSUM evacuation + output DMA overlap the
    next bank's matmuls.  The final bank is accumulated as two 256-column
    halves: half A finishes ~4.5us before the end, hiding its evacuation and
    output DMA entirely; only half B's short tail remains after the last
    matmul.

DMA plan: x tiles 1..7 ride the otherwise-idle GpSimd SWDGE queue right from
the start (per-tile semaphores, so each tanh fires as its tile lands), the
weight stream owns the Sync queue, and outputs go out on the Scalar queue.
This keeps all power planes ready ~10us before Phase B needs them.
"""

import numpy as np
import ml_dtypes

B, I, O, D1 = 4096, 1024, 1024, 8
NCORES = 8
BS = B // NCORES       # batch rows per core (moving free dim of each matmul)
IC = I // 128          # i chunks (contraction tiles per power plane)
OC = O // 128          # o chunks (output partition tiles)
NK = D1 - 1            # power planes k = 1..7
NJ = IC * NK           # K-steps per output tile
NJA = 14               # phase-A K-steps (covers planes of i-chunks 0..1)

# (oc, j) consumption order of the 448 stationary weight tiles
SEQ = [(oc, j) for j in range(NJA) for oc in range(OC)] + \
      [(oc, j) for oc in range(OC) for j in range(NJA, NJ)]
# weight-DMA chunk sizes (tiles): phase A starts fine-grained (the first
# chunk gates the first matmul) then coarsens; phase B is one 42-tile chunk
# per output group, which both minimizes PE semaphore waits and keeps the
# whole group resident so the last group can be swept twice (half A/half B).
_SIZES = [4, 4, 8, 16, 16, 16, 24, 24] + [NJ - NJA] * OC
CHUNKS = []
_s = 0
for _sz in _SIZES:
    CHUNKS.append((_s, _sz))
    _s += _sz
assert _s == OC * NJ
_NA = len(_SIZES) - OC                       # number of phase-A chunks

# chunk index whose last matmul completes group oc (phase B: 1 chunk/group)
GROUP_END_CHUNK = [_NA + oc for oc in range(OC)]

_GRAPH = None
LAST_RESULT = None     # BassKernelResults of the most recent run (for test.py)

# split the final output group's accumulation into two 256-col halves so
# half A's evac+store hide under half B's matmuls
SPLIT7 = False

# weight-chunk SBUF ring slots: deep enough that the sync sequencer's
# per-chunk descriptor generation (0.6-3.3us each, run-to-run variable)
# starts early enough for phase-B chunks to land before the PE reaches them.
CW_BUFS = 6


def _build_graph_raw():
    """Raw bacc build: manual per-engine streams + semaphores.  Saves the
    Tile exit drain + double all-engine barrier (~9us) and waits only once
    per weight chunk on the PE instead of per matmul."""
    import concourse.bass as bass
    from concourse import bacc, mybir

    nc = bacc.Bacc("TRN2", target_bir_lowering=False, debug=False,
                   num_devices=NCORES, monotonic_sem_count=0)
    f32 = mybir.dt.float32
    bf16 = mybir.dt.bfloat16

    xT = nc.dram_tensor("xT", [I, BS], bf16, kind="ExternalInput").ap()
    cw = nc.dram_tensor("cw", [128, OC * NJ * 128], bf16,
                        kind="ExternalInput").ap()
    bias = nc.dram_tensor("bias", [128, OC], f32, kind="ExternalInput").ap()
    yT = nc.dram_tensor("yT", [O, BS], f32, kind="ExternalOutput").ap()

    max_chunk = max(sz for _, sz in CHUNKS)
    xin = [nc.alloc_sbuf_tensor(f"xin{i}", [128, BS], bf16).ap()
           for i in range(IC)]
    planes = [nc.alloc_sbuf_tensor(f"pl{j}", [128, BS], bf16).ap()
              for j in range(NJ)]
    cwbuf = [nc.alloc_sbuf_tensor(f"cwb{i}", [128, max_chunk * 128],
                                  bf16).ap()
             for i in range(CW_BUFS)]
    # never written: garbage operand for PE p-state warm-up matmuls
    warm = nc.alloc_sbuf_tensor("warm", [128, 256], bf16).ap()
    warm2 = nc.alloc_sbuf_tensor("warm2", [128, BS], bf16).ap()
    bias_t = nc.alloc_sbuf_tensor("biasb", [128, OC], f32).ap()
    ot = [nc.alloc_sbuf_tensor(f"ot{i}", [128, BS], f32).ap()
          for i in range(2)]
    ps = [nc.alloc_psum_tensor(f"ps{i}", [128, BS], f32).ap()
          for i in range(OC)]
    HB = BS // 2
    ps7a, ps7b = ps[OC - 1][:, 0:HB], ps[OC - 1][:, HB:BS]

    from contextlib import ExitStack
    with ExitStack() as stack:
        # gpsimd issues only early DMAs whose completions are consumed mid-
        # kernel, so its expensive end-of-block dge_drain can be skipped
        block = stack.enter_context(nc.Block(no_gpsimd_drain=True))
        # DMA completion increments land as 16 per-slice +1s, and slices of
        # different in-flight DMAs interleave -- so a semaphore may only be
        # waited at "all DMAs issued on it so far" thresholds.  The weight
        # stream round-robins CW_BUFS semaphores (slot ring ensures only one
        # in-flight DMA per sem); x tiles get one sem each; output slots two.
        cw_dma = [stack.enter_context(nc.semaphore(f"cw_dma{r}"))
                  for r in range(CW_BUFS)]
        # xin0 gates the first tanh and rides the sync queue ahead of the
        # weight stream; xins 1..7 go on gpsimd SWDGE with per-tile sems
        # (SWDGE and HWDGE DMAs may not mix on a sem)
        xin0_dma = stack.enter_context(nc.semaphore("xin0_dma"))
        xr_dma = [stack.enter_context(nc.semaphore(f"xr_dma{i}"))
                  for i in range(IC - 1)]
        bias_dma = stack.enter_context(nc.semaphore("bias_dma"))
        out_dma = [stack.enter_context(nc.semaphore(f"out_dma{r}"))
                   for r in range(2)]
        act_pl = stack.enter_context(nc.semaphore("act_pl"))
        dve_pl = stack.enter_context(nc.semaphore("dve_pl"))
        pe_ch = stack.enter_context(nc.semaphore("pe_ch"))
        pe_half = stack.enter_context(nc.semaphore("pe_half"))
        act_ev = stack.enter_context(nc.semaphore("act_ev"))

        @block.sync
        def _(eng: bass.BassEngine):
            # xin0 first: it gates the whole plane pipeline
            eng.dma_start(out=xin[0][:], in_=xT[0:128, :]
                          ).then_inc(xin0_dma, 16)
            for ci, (s0, size) in enumerate(CHUNKS):
                if ci >= CW_BUFS:
                    eng.wait_ge(pe_ch, ci - CW_BUFS + 1)
                eng.dma_start(
                    out=cwbuf[ci % CW_BUFS][:, :size * 128],
                    in_=cw[:, s0 * 128:(s0 + size) * 128],
                ).then_inc(cw_dma[ci % CW_BUFS], 16)

        @block.gpsimd
        def _(eng: bass.BassEngine):
            # x tiles 1..7 + bias on the otherwise-idle SWDGE queue; issued
            # immediately so every tanh input is resident by ~15us
            for i in range(1, IC):
                eng.dma_start(out=xin[i][:], in_=xT[i * 128:(i + 1) * 128, :]
                              ).then_inc(xr_dma[i - 1], 16)
            eng.dma_start(out=bias_t[:], in_=bias[:]).then_inc(bias_dma, 16)

        @block.scalar
        def _(eng: bass.BassEngine):
            eng.wait_ge(xin0_dma, 16)
            eng.activation(planes[0][:], xin[0][:],
                           mybir.ActivationFunctionType.Tanh
                           ).then_inc(act_pl, 1)
            for i in range(1, IC):
                eng.wait_ge(xr_dma[i - 1], 16)
                eng.activation(planes[i * NK][:], xin[i][:],
                               mybir.ActivationFunctionType.Tanh
                               ).then_inc(act_pl, 1)
            eng.wait_ge(bias_dma, 16)
            ev = 0
            for oc in range(OC - 1):
                eng.wait_ge(pe_ch, GROUP_END_CHUNK[oc] + 1)
                if oc >= 2:
                    eng.wait_ge(out_dma[oc % 2], 16 * (oc // 2))
                eng.activation(ot[oc % 2][:], ps[oc][:],
                               mybir.ActivationFunctionType.Identity,
                               bias=bias_t[:, oc:oc + 1]
                               ).then_inc(act_ev, 1)
                ev += 1
                eng.wait_ge(act_ev, ev)
                eng.dma_start(
                    out=yT[oc * 128:(oc + 1) * 128, :],
                    in_=ot[oc % 2][:]
                ).then_inc(out_dma[oc % 2], 16)
            # last group: half A finished NJ-NJA matmuls early -- its evac +
            # store run under half B's matmuls; only half B's tail is serial
            o0 = (OC - 1) * 128
            if SPLIT7:
                eng.wait_ge(pe_half, 1)
            else:
                eng.wait_ge(pe_ch, len(CHUNKS))
            eng.wait_ge(out_dma[1], 16 * ((OC - 1) // 2))
            eng.activation(ot[1][:, 0:HB], ps7a,
                           mybir.ActivationFunctionType.Identity,
                           bias=bias_t[:, OC - 1:OC]).then_inc(act_ev, 1)
            ev += 1
            eng.wait_ge(act_ev, ev)
            eng.dma_start(out=yT[o0:o0 + 128, 0:HB], in_=ot[1][:, 0:HB]
                          ).then_inc(out_dma[1], 16)
            if SPLIT7:
                eng.wait_ge(pe_ch, len(CHUNKS))
            eng.activation(ot[1][:, HB:BS], ps7b,
                           mybir.ActivationFunctionType.Identity,
                           bias=bias_t[:, OC - 1:OC]).then_inc(act_ev, 1)
            ev += 1
            eng.wait_ge(act_ev, ev)
            eng.dma_start(out=yT[o0:o0 + 128, HB:BS], in_=ot[1][:, HB:BS]
                          ).then_inc(out_dma[1], 16)
            # no final out-DMA waits: the runtime drains DMA queues at NEFF
            # end, and nothing reuses ot afterwards

        @block.vector
        def _(eng: bass.BassEngine):
            for i in range(IC):
                eng.wait_ge(act_pl, i + 1)
                for k1 in range(1, NK):
                    if k1 >= 2:
                        # same-engine RAW still needs a sem wait (deep
                        # pipeline, no interlock)
                        eng.wait_ge(dve_pl, i * (NK - 1) + k1 - 1)
                    eng.tensor_mul(planes[i * NK + k1][:],
                                   planes[i * NK + k1 - 1][:],
                                   planes[i * NK][:]
                                   ).then_inc(dve_pl, 1)

        @block.tensor
        def _(eng: bass.BassEngine):
            # p-state warm-up on garbage inputs: no waits, runs while the
            # first x tile + weight chunk DMAs are in flight, so the real
            # stream starts at (or near) full clock
            for _ in range(16):
                eng.matmul(ps[0][:], warm2[:, 0:128], warm2[:],
                           start=True, stop=True)
            doneA = [0] * OC
            doneB = 0
            seen_act = seen_dve = 0
            for ci, (s0, size) in enumerate(CHUNKS):
                # attach all of the chunk's waits to its first matmul --
                # the move_matmul_waits_to_ldweights compile pass hoists
                # them onto the LDWEIGHTS, keeping the PE's 64-deep
                # reorder window free to pull later weight loads ahead
                js = [SEQ[s][1] for s in range(s0, s0 + size)]
                need_act = max((j // NK + 1 for j in js if j % NK == 0),
                               default=0)
                need_dve = max((j // NK * (NK - 1) + j % NK
                                for j in js if j % NK != 0), default=0)
                if need_act > seen_act:
                    eng.wait_ge(act_pl, need_act)
                    seen_act = need_act
                if need_dve > seen_dve:
                    eng.wait_ge(dve_pl, need_dve)
                    seen_dve = need_dve
                last7 = (ci == len(CHUNKS) - 1) and SPLIT7   # two sweeps
                for t in range(size):
                    oc, j = SEQ[s0 + t]
                    cwap = cwbuf[ci % CW_BUFS][:, t * 128:(t + 1) * 128]
                    if oc < OC - 1 or not SPLIT7:
                        mm = eng.matmul(ps[oc][:], cwap, planes[j][:],
                                        start=(doneA[oc] == 0),
                                        stop=(doneA[oc] == NJ - 1))
                        doneA[oc] += 1
                    else:
                        mm = eng.matmul(ps7a, cwap, planes[j][:, 0:HB],
                                        start=(doneA[oc] == 0),
                                        stop=(doneA[oc] == NJ - 1))
                        if doneA[oc] == NJ - 1:
                            mm.then_inc(pe_half, 1)
                        doneA[oc] += 1
                        if not last7:
                            # phase A: same stationary tile, second half
                            eng.matmul(ps7b, cwap, planes[j][:, HB:BS],
                                       start=(doneB == 0), stop=False)
                            doneB += 1
                    if t == 0:
                        mm._wait_ge(cw_dma[ci % CW_BUFS],
                                    16 * (ci // CW_BUFS + 1))
                    if t == size - 1 and not last7:
                        mm.then_inc(pe_ch, 1)
                if last7:
                    # second sweep: half B of the final group (re-loads the
                    # 42 stationary tiles; LDWEIGHTS hides under the 256-col
                    # matmuls)
                    for t in range(size):
                        _, j = SEQ[s0 + t]
                        mm = eng.matmul(
                            ps7b,
                            cwbuf[ci % CW_BUFS][:, t * 128:(t + 1) * 128],
                            planes[j][:, HB:BS],
                            start=(doneB == 0), stop=(doneB == NJ - 1))
                        doneB += 1
                        if t == size - 1:
                            mm.then_inc(pe_ch, 1)
            assert all(d == NJ for d in doneA)
            assert doneB == (NJ if SPLIT7 else 0)

    nc.compile()
    return nc


def _get_graph():
    global _GRAPH
    if _GRAPH is None:
        _GRAPH = _build_graph_raw()
    return _GRAPH


def _host_prep(a, q, coeffs):
    """Fold the polynomial basis change into the weights (float64 on host)."""
    # c[d, k]: P_d(t) = sum_k c[d, k] * t^k, from the three-term recurrence
    c = np.zeros((D1, D1), np.float64)
    c[0, 0] = 1.0
    if D1 > 1:
        c[1, 1] = 1.0
        c[1, 0] = -a
    for n in range(2, D1):
        c[n, 1:] += c[n - 1, :-1]
        c[n, :] -= (a + q ** n) * c[n - 1, :]
        c[n, :] -= a * q ** (n - 1) * c[n - 2, :]

    Cf = (coeffs.reshape(-1, D1).astype(np.float64) @ c).reshape(I, O, D1)
    bias = Cf[:, :, 0].sum(axis=0).astype(np.float32)                # [O]
    Ck = Cf[:, :, 1:].astype(np.float32).astype(ml_dtypes.bfloat16)  # [I,O,NK]

    # stationary tile for (oc, j=ic*NK+k1): [128 i-part, 128 o-col] slice
    t = Ck.reshape(IC, 128, OC, 128, NK)            # [ic, p, oc, ol, k1]
    X = np.ascontiguousarray(t.transpose(2, 0, 4, 1, 3)) \
          .reshape(OC, NJ, 128, 128)                # [oc, j, p, ol]
    oc_idx = np.array([oc for oc, _ in SEQ])
    j_idx = np.array([j for _, j in SEQ])
    seq_tiles = X[oc_idx, j_idx]                    # [448, p, ol]
    cw_dev = np.ascontiguousarray(
        seq_tiles.transpose(1, 0, 2)).reshape(128, OC * NJ * 128)
    bias_dev = np.ascontiguousarray(bias.reshape(OC, 128).T)  # [128, OC]
    return cw_dev, bias_dev


def _ensure_axon_hooks_importable():
    """run_bass_kernel_spmd imports antenv.axon_hooks when BASS_TRACE is
    set; some images lack that module.  Register a no-op fallback so a
    trace request degrades to a warning instead of an ImportError."""
    import sys
    import types
    if "antenv.axon_hooks" in sys.modules:
        return
    try:
        import antenv.axon_hooks  # noqa: F401
    except ImportError:
        mod = types.ModuleType("antenv.axon_hooks")
        state = {"hook": None}
        mod.set_axon_ntff_profile_hook = \
            lambda h: state.__setitem__("hook", h)
        mod.get_axon_ntff_profile_hook = lambda: state["hook"]
        sys.modules["antenv.axon_hooks"] = mod
        try:
            import antenv
            antenv.axon_hooks = mod
        except ImportError:
            pass


def kernel(x, a, q, coeffs):
    global LAST_RESULT
    _ensure_axon_hooks_importable()
    from concourse.bass_utils import run_bass_kernel_spmd

    x = np.ascontiguousarray(np.asarray(x, dtype=np.float32))
    coeffs = np.ascontiguousarray(np.asarray(coeffs, dtype=np.float32))
    a_val = float(np.asarray(a).reshape(-1)[0])
    q_val = float(np.asarray(q).reshape(-1)[0])

    cw_dev, bias_dev = _host_prep(a_val, q_val, coeffs)
    xs = x.reshape(NCORES, BS, I).transpose(0, 2, 1)  # [core, I, BS]
    xs = xs.astype(ml_dtypes.bfloat16)

    in_maps = [{
        "xT": np.ascontiguousarray(xs[c]),
        "cw": cw_dev,
        "bias": bias_dev,
    } for c in range(NCORES)]

    nc = _get_graph()
    res = run_bass_kernel_spmd(nc, in_maps, core_ids=list(range(NCORES)))
    LAST_RESULT = res

    shards = [np.asarray(res.results[c]["yT"]).T for c in range(NCORES)]
    return np.ascontiguousarray(np.concatenate(shards, axis=0),
                                dtype=np.float32)


if __name__ == "__main__":
    rng = np.random.default_rng(0)
    inputs = {
        "x": rng.standard_normal((B, I), dtype=np.float32),
        "a": np.zeros((1,), np.float32),
        "q": np.ones((1,), np.float32),
        "coeffs": rng.standard_normal((I, O, D1), dtype=np.float32)
        / (I * D1),
    }
    y = kernel(**inputs)
    print("out", y.shape, y.dtype, float(np.abs(y).mean()))


# revision 17
# speedup vs baseline: 1.0777x; 1.0076x over previous
"""Al-Salam-Carlitz KAN layer on 8 TRN2 NeuronCores.

Math: y[b,o] = sum_{i,d} P_d(tanh(x[b,i])) * coeffs[i,o,d], where P_d are the
Al-Salam-Carlitz polynomials given by a three-term recurrence in scalars a, q.
Each P_d is a degree-d polynomial in t = tanh(x), so on the host we fold the
(D+1)x(D+1) basis-change matrix into coeffs:

    y[b,o] = bias[o] + sum_{k=1..D} sum_i t[b,i]^k * Cf[i,o,k]

with bias[o] = sum_i Cf[i,o,0] (the k=0 column times t^0 == 1).  This removes
1/8 of the matmul work and leaves the device with: tanh, a bf16 power chain,
and a K=7*1024 contraction done as 448 TensorE matmuls per core.

Sharding: data-parallel over batch (4096 -> 8 x 512).  Each core receives its
x-shard pre-transposed ([I, 512] in bf16, so the contraction dim lands on SBUF
partitions), the folded weights (bf16, pre-laid-out in exact consumption
order for contiguous chunked DMA), and the bias.  No collectives; the host
concatenates the 8 output shards.

Matmul schedule (one core): 8 output tiles yT[oc] = [128 o, 512 b], each
accumulating 56 K-steps in PSUM bank oc.
  Warm-up: ~16 small dummy matmuls on garbage SBUF ramp the PE out of its
    low-power p-state (first ~3-6us run at 1.2 instead of 2.4 GHz) while the
    first x-tile DMA + tanh are still in flight.
  Phase A (j = 0..13): for each j, one matmul into every bank -- consumption
    of power planes is 8x slower than production, so the PE never stalls on
    the tanh/power chain during ramp-up.
  Phase B (oc = 0..7): finish each bank's remaining 42 K-steps back-to-back,
    so banks complete staggered and PSUM evacuation + output DMA overlap the
    next bank's matmuls.  The final bank is accumulated as two 256-column
    halves: half A finishes ~4.5us before the end, hiding its evacuation and
    output DMA entirely; only half B's short tail remains after the last
    matmul.

DMA plan: x tiles 1..7 ride the otherwise-idle GpSimd SWDGE queue right from
the start (per-tile semaphores, so each tanh fires as its tile lands), the
weight stream owns the Sync queue, and outputs go out on the Scalar queue.
This keeps all power planes ready ~10us before Phase B needs them.
"""

import numpy as np
import ml_dtypes

B, I, O, D1 = 4096, 1024, 1024, 8
NCORES = 8
BS = B // NCORES       # batch rows per core (moving free dim of each matmul)
IC = I // 128          # i chunks (contraction tiles per power plane)
OC = O // 128          # o chunks (output partition tiles)
NK = D1 - 1            # power planes k = 1..7
NJ = IC * NK           # K-steps per output tile
NJA = 14               # phase-A K-steps (covers planes of i-chunks 0..1)

# (oc, j) consumption order of the 448 stationary weight tiles
SEQ = [(oc, j) for j in range(NJA) for oc in range(OC)] + \
      [(oc, j) for oc in range(OC) for j in range(NJA, NJ)]
# weight-DMA chunk sizes (tiles): phase A starts fine-grained (the first
# chunk gates the first matmul) then coarsens; phase B is one 42-tile chunk
# per output group, which both minimizes PE semaphore waits and keeps the
# whole group resident.  The first N_GP chunks ride the GpSimd SWDGE queue
# (in parallel with xin0 on the Sync queue) so the PE's ramp is gated only
# by the first tanh, not by the weight stream.
_SIZES = [4, 4, 8, 8, 8, 16, 16, 16, 16, 16] + [NJ - NJA] * OC
N_GP = 3               # leading weight chunks issued from the GpSimd queue
CHUNKS = []
_s = 0
for _sz in _SIZES:
    CHUNKS.append((_s, _sz))
    _s += _sz
assert _s == OC * NJ
_NA = len(_SIZES) - OC                       # number of phase-A chunks

# chunk index whose last matmul completes group oc (phase B: 1 chunk/group)
GROUP_END_CHUNK = [_NA + oc for oc in range(OC)]

_GRAPH = None
LAST_RESULT = None     # BassKernelResults of the most recent run (for test.py)

# split the final output group's accumulation into two 256-col halves so
# half A's evac+store hide under half B's matmuls
SPLIT7 = False

# weight-chunk SBUF ring slots: deep enough that the sync sequencer's
# per-chunk descriptor generation (0.6-3.3us each, run-to-run variable)
# starts early enough for phase-B chunks to land before the PE reaches them.
CW_BUFS = 6


def _build_graph_raw():
    """Raw bacc build: manual per-engine streams + semaphores.  Saves the
    Tile exit drain + double all-engine barrier (~9us) and waits only once
    per weight chunk on the PE instead of per matmul."""
    import concourse.bass as bass
    from concourse import bacc, mybir

    nc = bacc.Bacc("TRN2", target_bir_lowering=False, debug=False,
                   num_devices=NCORES, monotonic_sem_count=0)
    f32 = mybir.dt.float32
    bf16 = mybir.dt.bfloat16

    xT = nc.dram_tensor("xT", [I, BS], bf16, kind="ExternalInput").ap()
    cw = nc.dram_tensor("cw", [128, OC * NJ * 128], bf16,
                        kind="ExternalInput").ap()
    bias = nc.dram_tensor("bias", [128, OC], f32, kind="ExternalInput").ap()
    yT = nc.dram_tensor("yT", [O, BS], f32, kind="ExternalOutput").ap()

    max_chunk = max(sz for _, sz in CHUNKS)
    xin = [nc.alloc_sbuf_tensor(f"xin{i}", [128, BS], bf16).ap()
           for i in range(IC)]
    planes = [nc.alloc_sbuf_tensor(f"pl{j}", [128, BS], bf16).ap()
              for j in range(NJ)]
    cwbuf = [nc.alloc_sbuf_tensor(f"cwb{i}", [128, max_chunk * 128],
                                  bf16).ap()
             for i in range(CW_BUFS)]
    # never written: garbage operand for PE p-state warm-up matmuls
    warm = nc.alloc_sbuf_tensor("warm", [128, 256], bf16).ap()
    warm2 = nc.alloc_sbuf_tensor("warm2", [128, BS], bf16).ap()
    bias_t = nc.alloc_sbuf_tensor("biasb", [128, OC], f32).ap()
    ot = [nc.alloc_sbuf_tensor(f"ot{i}", [128, BS], f32).ap()
          for i in range(2)]
    ps = [nc.alloc_psum_tensor(f"ps{i}", [128, BS], f32).ap()
          for i in range(OC)]
    HB = BS // 2
    ps7a, ps7b = ps[OC - 1][:, 0:HB], ps[OC - 1][:, HB:BS]

    from contextlib import ExitStack
    with ExitStack() as stack:
        # gpsimd issues only early DMAs whose completions are consumed mid-
        # kernel, so its expensive end-of-block dge_drain can be skipped
        block = stack.enter_context(nc.Block(no_gpsimd_drain=True))
        # DMA completion increments land as 16 per-slice +1s, and slices of
        # different in-flight DMAs interleave -- so a semaphore may only be
        # waited at "all DMAs issued on it so far" thresholds.  The weight
        # stream round-robins CW_BUFS semaphores (slot ring ensures only one
        # in-flight DMA per sem); x tiles get one sem each; output slots two.
        cw_dma = [stack.enter_context(nc.semaphore(f"cw_dma{r}"))
                  for r in range(CW_BUFS)]
        # SWDGE and HWDGE may not share a sem: the gpsimd-issued leading
        # chunks get dedicated sems even though they share the buffer ring
        cwg = [stack.enter_context(nc.semaphore(f"cwg{r}"))
               for r in range(N_GP)]
        # xin0 gates the first tanh and rides the sync queue ahead of the
        # weight stream; xins 1..7 go on gpsimd SWDGE with per-tile sems
        # (SWDGE and HWDGE DMAs may not mix on a sem)
        xin0_dma = stack.enter_context(nc.semaphore("xin0_dma"))
        xr_dma = [stack.enter_context(nc.semaphore(f"xr_dma{i}"))
                  for i in range(IC - 1)]
        bias_dma = stack.enter_context(nc.semaphore("bias_dma"))
        out_dma = [stack.enter_context(nc.semaphore(f"out_dma{r}"))
                   for r in range(2)]
        act_pl = stack.enter_context(nc.semaphore("act_pl"))
        dve_pl = stack.enter_context(nc.semaphore("dve_pl"))
        pe_ch = stack.enter_context(nc.semaphore("pe_ch"))
        pe_half = stack.enter_context(nc.semaphore("pe_half"))
        act_ev = stack.enter_context(nc.semaphore("act_ev"))

        @block.sync
        def _(eng: bass.BassEngine):
            # xin0 first: it gates the whole plane pipeline
            eng.dma_start(out=xin[0][:], in_=xT[0:128, :]
                          ).then_inc(xin0_dma, 16)
            for ci, (s0, size) in enumerate(CHUNKS):
                if ci < N_GP:
                    continue               # leading chunks ride gpsimd
                if ci >= CW_BUFS:
                    eng.wait_ge(pe_ch, ci - CW_BUFS + 1)
                eng.dma_start(
                    out=cwbuf[ci % CW_BUFS][:, :size * 128],
                    in_=cw[:, s0 * 128:(s0 + size) * 128],
                ).then_inc(cw_dma[ci % CW_BUFS], 16)
            # output stores: the evac->store handoff runs here so the DMA
            # issue cost (~0.6us each) never serializes with the next evac
            # on the Scalar queue
            for oc in range(OC - 1):
                eng.wait_ge(act_ev, oc + 1)
                eng.dma_start(
                    out=yT[oc * 128:(oc + 1) * 128, :],
                    in_=ot[oc % 2][:]
                ).then_inc(out_dma[oc % 2], 16)
            o0 = (OC - 1) * 128
            eng.wait_ge(act_ev, OC)
            eng.dma_start(out=yT[o0:o0 + 128, 0:HB], in_=ot[1][:, 0:HB]
                          ).then_inc(out_dma[1], 16)
            eng.wait_ge(act_ev, OC + 1)
            eng.dma_start(out=yT[o0:o0 + 128, HB:BS], in_=ot[1][:, HB:BS]
                          ).then_inc(out_dma[1], 16)

        @block.gpsimd
        def _(eng: bass.BassEngine):
            # leading weight chunks + x tiles 1..7 + bias on the otherwise-
            # idle SWDGE queue, in consumption order
            for ci in range(N_GP):
                s0, size = CHUNKS[ci]
                eng.dma_start(
                    out=cwbuf[ci % CW_BUFS][:, :size * 128],
                    in_=cw[:, s0 * 128:(s0 + size) * 128],
                ).then_inc(cwg[ci], 16)
            for i in range(1, IC):
                eng.dma_start(out=xin[i][:], in_=xT[i * 128:(i + 1) * 128, :]
                              ).then_inc(xr_dma[i - 1], 16)
            eng.dma_start(out=bias_t[:], in_=bias[:]).then_inc(bias_dma, 16)

        @block.scalar
        def _(eng: bass.BassEngine):
            eng.wait_ge(xin0_dma, 16)
            eng.activation(planes[0][:], xin[0][:],
                           mybir.ActivationFunctionType.Tanh
                           ).then_inc(act_pl, 1)
            for i in range(1, IC):
                eng.wait_ge(xr_dma[i - 1], 16)
                eng.activation(planes[i * NK][:], xin[i][:],
                               mybir.ActivationFunctionType.Tanh
                               ).then_inc(act_pl, 1)
            eng.wait_ge(bias_dma, 16)
            for oc in range(OC - 1):
                eng.wait_ge(pe_ch, GROUP_END_CHUNK[oc] + 1)
                if oc >= 2:
                    eng.wait_ge(out_dma[oc % 2], 16 * (oc // 2))
                eng.activation(ot[oc % 2][:], ps[oc][:],
                               mybir.ActivationFunctionType.Identity,
                               bias=bias_t[:, oc:oc + 1]
                               ).then_inc(act_ev, 1)
            # last group: two half-column evacs so the first store issues
            # (on the Sync queue) while the second half is still evacuating
            eng.wait_ge(pe_ch, len(CHUNKS))
            eng.wait_ge(out_dma[1], 16 * ((OC - 1) // 2))
            eng.activation(ot[1][:, 0:HB], ps7a,
                           mybir.ActivationFunctionType.Identity,
                           bias=bias_t[:, OC - 1:OC]).then_inc(act_ev, 1)
            eng.activation(ot[1][:, HB:BS], ps7b,
                           mybir.ActivationFunctionType.Identity,
                           bias=bias_t[:, OC - 1:OC]).then_inc(act_ev, 1)
            # no final out-DMA waits: the runtime drains DMA queues at NEFF
            # end, and nothing reuses ot afterwards

        @block.vector
        def _(eng: bass.BassEngine):
            for i in range(IC):
                eng.wait_ge(act_pl, i + 1)
                for k1 in range(1, NK):
                    if k1 >= 2:
                        # same-engine RAW still needs a sem wait (deep
                        # pipeline, no interlock)
                        eng.wait_ge(dve_pl, i * (NK - 1) + k1 - 1)
                    eng.tensor_mul(planes[i * NK + k1][:],
                                   planes[i * NK + k1 - 1][:],
                                   planes[i * NK][:]
                                   ).then_inc(dve_pl, 1)

        @block.tensor
        def _(eng: bass.BassEngine):
            # p-state warm-up on garbage inputs: no waits, runs while the
            # first x tile + weight chunk DMAs are in flight, so the real
            # stream starts at (or near) full clock
            for _ in range(10):
                eng.matmul(ps[0][:], warm2[:, 0:128], warm2[:],
                           start=True, stop=True)
            doneA = [0] * OC
            doneB = 0
            seen_act = seen_dve = 0
            sem_uses = [0] * CW_BUFS   # HWDGE waits per ring sem
            for ci, (s0, size) in enumerate(CHUNKS):
                # attach all of the chunk's waits to its first matmul --
                # the move_matmul_waits_to_ldweights compile pass hoists
                # them onto the LDWEIGHTS, keeping the PE's 64-deep
                # reorder window free to pull later weight loads ahead
                js = [SEQ[s][1] for s in range(s0, s0 + size)]
                need_act = max((j // NK + 1 for j in js if j % NK == 0),
                               default=0)
                need_dve = max((j // NK * (NK - 1) + j % NK
                                for j in js if j % NK != 0), default=0)
                if need_act > seen_act:
                    eng.wait_ge(act_pl, need_act)
                    seen_act = need_act
                if need_dve > seen_dve:
                    eng.wait_ge(dve_pl, need_dve)
                    seen_dve = need_dve
                last7 = (ci == len(CHUNKS) - 1) and SPLIT7   # two sweeps
                for t in range(size):
                    oc, j = SEQ[s0 + t]
                    cwap = cwbuf[ci % CW_BUFS][:, t * 128:(t + 1) * 128]
                    if oc < OC - 1 or not SPLIT7:
                        mm = eng.matmul(ps[oc][:], cwap, planes[j][:],
                                        start=(doneA[oc] == 0),
                                        stop=(doneA[oc] == NJ - 1))
                        doneA[oc] += 1
                    else:
                        mm = eng.matmul(ps7a, cwap, planes[j][:, 0:HB],
                                        start=(doneA[oc] == 0),
                                        stop=(doneA[oc] == NJ - 1))
                        if doneA[oc] == NJ - 1:
                            mm.then_inc(pe_half, 1)
                        doneA[oc] += 1
                        if not last7:
                            # phase A: same stationary tile, second half
                            eng.matmul(ps7b, cwap, planes[j][:, HB:BS],
                                       start=(doneB == 0), stop=False)
                            doneB += 1
                    if t == 0:
                        if ci < N_GP:
                            mm._wait_ge(cwg[ci], 16)
                        else:
                            sem_uses[ci % CW_BUFS] += 1
                            mm._wait_ge(cw_dma[ci % CW_BUFS],
                                        16 * sem_uses[ci % CW_BUFS])
                    if t == size - 1 and not last7:
                        mm.then_inc(pe_ch, 1)
                if last7:
                    # second sweep: half B of the final group (re-loads the
                    # 42 stationary tiles; LDWEIGHTS hides under the 256-col
                    # matmuls)
                    for t in range(size):
                        _, j = SEQ[s0 + t]
                        mm = eng.matmul(
                            ps7b,
                            cwbuf[ci % CW_BUFS][:, t * 128:(t + 1) * 128],
                            planes[j][:, HB:BS],
                            start=(doneB == 0), stop=(doneB == NJ - 1))
                        doneB += 1
                        if t == size - 1:
                            mm.then_inc(pe_ch, 1)
            assert all(d == NJ for d in doneA)
            assert doneB == (NJ if SPLIT7 else 0)

    nc.compile()
    return nc


def _get_graph():
    global _GRAPH
    if _GRAPH is None:
        _GRAPH = _build_graph_raw()
    return _GRAPH


def _host_prep(a, q, coeffs):
    """Fold the polynomial basis change into the weights (float64 on host)."""
    # c[d, k]: P_d(t) = sum_k c[d, k] * t^k, from the three-term recurrence
    c = np.zeros((D1, D1), np.float64)
    c[0, 0] = 1.0
    if D1 > 1:
        c[1, 1] = 1.0
        c[1, 0] = -a
    for n in range(2, D1):
        c[n, 1:] += c[n - 1, :-1]
        c[n, :] -= (a + q ** n) * c[n - 1, :]
        c[n, :] -= a * q ** (n - 1) * c[n - 2, :]

    Cf = (coeffs.reshape(-1, D1).astype(np.float64) @ c).reshape(I, O, D1)
    bias = Cf[:, :, 0].sum(axis=0).astype(np.float32)                # [O]
    Ck = Cf[:, :, 1:].astype(np.float32).astype(ml_dtypes.bfloat16)  # [I,O,NK]

    # stationary tile for (oc, j=ic*NK+k1): [128 i-part, 128 o-col] slice
    t = Ck.reshape(IC, 128, OC, 128, NK)            # [ic, p, oc, ol, k1]
    X = np.ascontiguousarray(t.transpose(2, 0, 4, 1, 3)) \
          .reshape(OC, NJ, 128, 128)                # [oc, j, p, ol]
    oc_idx = np.array([oc for oc, _ in SEQ])
    j_idx = np.array([j for _, j in SEQ])
    seq_tiles = X[oc_idx, j_idx]                    # [448, p, ol]
    cw_dev = np.ascontiguousarray(
        seq_tiles.transpose(1, 0, 2)).reshape(128, OC * NJ * 128)
    bias_dev = np.ascontiguousarray(bias.reshape(OC, 128).T)  # [128, OC]
    return cw_dev, bias_dev


def _ensure_axon_hooks_importable():
    """run_bass_kernel_spmd imports antenv.axon_hooks when BASS_TRACE is
    set; some images lack that module.  Register a no-op fallback so a
    trace request degrades to a warning instead of an ImportError."""
    import sys
    import types
    if "antenv.axon_hooks" in sys.modules:
        return
    try:
        import antenv.axon_hooks  # noqa: F401
    except ImportError:
        mod = types.ModuleType("antenv.axon_hooks")
        state = {"hook": None}
        mod.set_axon_ntff_profile_hook = \
            lambda h: state.__setitem__("hook", h)
        mod.get_axon_ntff_profile_hook = lambda: state["hook"]
        sys.modules["antenv.axon_hooks"] = mod
        try:
            import antenv
            antenv.axon_hooks = mod
        except ImportError:
            pass


def kernel(x, a, q, coeffs):
    global LAST_RESULT
    _ensure_axon_hooks_importable()
    from concourse.bass_utils import run_bass_kernel_spmd

    x = np.ascontiguousarray(np.asarray(x, dtype=np.float32))
    coeffs = np.ascontiguousarray(np.asarray(coeffs, dtype=np.float32))
    a_val = float(np.asarray(a).reshape(-1)[0])
    q_val = float(np.asarray(q).reshape(-1)[0])

    cw_dev, bias_dev = _host_prep(a_val, q_val, coeffs)
    xs = x.reshape(NCORES, BS, I).transpose(0, 2, 1)  # [core, I, BS]
    xs = xs.astype(ml_dtypes.bfloat16)

    in_maps = [{
        "xT": np.ascontiguousarray(xs[c]),
        "cw": cw_dev,
        "bias": bias_dev,
    } for c in range(NCORES)]

    nc = _get_graph()
    res = run_bass_kernel_spmd(nc, in_maps, core_ids=list(range(NCORES)))
    LAST_RESULT = res

    shards = [np.asarray(res.results[c]["yT"]).T for c in range(NCORES)]
    return np.ascontiguousarray(np.concatenate(shards, axis=0),
                                dtype=np.float32)


if __name__ == "__main__":
    rng = np.random.default_rng(0)
    inputs = {
        "x": rng.standard_normal((B, I), dtype=np.float32),
        "a": np.zeros((1,), np.float32),
        "q": np.ones((1,), np.float32),
        "coeffs": rng.standard_normal((I, O, D1), dtype=np.float32)
        / (I * D1),
    }
    y = kernel(**inputs)
    print("out", y.shape, y.dtype, float(np.abs(y).mean()))


# revision 46
# speedup vs baseline: 1.0797x; 1.0018x over previous
"""Al-Salam-Carlitz KAN layer on 8 TRN2 NeuronCores.

Math: y[b,o] = sum_{i,d} P_d(tanh(x[b,i])) * coeffs[i,o,d], where P_d are the
Al-Salam-Carlitz polynomials given by a three-term recurrence in scalars a, q.
Each P_d is a degree-d polynomial in t = tanh(x), so on the host we fold the
(D+1)x(D+1) basis-change matrix into coeffs:

    y[b,o] = bias[o] + sum_{k=1..D} sum_i t[b,i]^k * Cf[i,o,k]

with bias[o] = sum_i Cf[i,o,0] (the k=0 column times t^0 == 1).  This removes
1/8 of the matmul work and leaves the device with: tanh, a bf16 power chain,
and a K=7*1024 contraction done as 448 TensorE matmuls per core.

Sharding: data-parallel over batch (4096 -> 8 x 512).  Each core receives its
x-shard pre-transposed ([I, 512] in bf16, so the contraction dim lands on SBUF
partitions), the folded weights (bf16, pre-laid-out in exact consumption
order for contiguous chunked DMA), and the bias.  No collectives; the host
concatenates the 8 output shards.

Matmul schedule (one core): 8 output tiles yT[oc] = [128 o, 512 b], each
accumulating 56 K-steps in PSUM bank oc.
  Warm-up: ~16 small dummy matmuls on garbage SBUF ramp the PE out of its
    low-power p-state (first ~3-6us run at 1.2 instead of 2.4 GHz) while the
    first x-tile DMA + tanh are still in flight.
  Phase A (j = 0..13): for each j, one matmul into every bank -- consumption
    of power planes is 8x slower than production, so the PE never stalls on
    the tanh/power chain during ramp-up.
  Phase B (oc = 0..7): finish each bank's remaining 42 K-steps back-to-back,
    so banks complete staggered and PSUM evacuation + output DMA overlap the
    next bank's matmuls.  The final bank is accumulated as two 256-column
    halves: half A finishes ~4.5us before the end, hiding its evacuation and
    output DMA entirely; only half B's short tail remains after the last
    matmul.

DMA plan: x tiles 1..7 ride the otherwise-idle GpSimd SWDGE queue right from
the start (per-tile semaphores, so each tanh fires as its tile lands), the
weight stream owns the Sync queue, and outputs go out on the Scalar queue.
This keeps all power planes ready ~10us before Phase B needs them.
"""

import numpy as np
import ml_dtypes

B, I, O, D1 = 4096, 1024, 1024, 8
NCORES = 8
BS = B // NCORES       # batch rows per core (moving free dim of each matmul)
IC = I // 128          # i chunks (contraction tiles per power plane)
OC = O // 128          # o chunks (output partition tiles)
NK = D1 - 1            # power planes k = 1..7
NJ = IC * NK           # K-steps per output tile
NJA = 14               # phase-A K-steps (covers planes of i-chunks 0..1)

# (oc, j) consumption order of the 448 stationary weight tiles
SEQ = [(oc, j) for j in range(NJA) for oc in range(OC)] + \
      [(oc, j) for oc in range(OC) for j in range(NJA, NJ)]
# weight-DMA chunk sizes (tiles): phase A starts fine-grained (the first
# chunk gates the first matmul) then coarsens; phase B is one 42-tile chunk
# per output group, which both minimizes PE semaphore waits and keeps the
# whole group resident.  The first N_GP chunks ride the GpSimd SWDGE queue
# (in parallel with xin0 on the Sync queue) so the PE's ramp is gated only
# by the first tanh, not by the weight stream.
_SIZES = [4, 4, 8, 8, 8, 16, 16, 16, 16, 16] + [NJ - NJA] * OC
N_GP = 2               # leading weight chunks issued from the GpSimd queue
CHUNKS = []
_s = 0
for _sz in _SIZES:
    CHUNKS.append((_s, _sz))
    _s += _sz
assert _s == OC * NJ
_NA = len(_SIZES) - OC                       # number of phase-A chunks

# chunk index whose last matmul completes group oc (phase B: 1 chunk/group)
GROUP_END_CHUNK = [_NA + oc for oc in range(OC)]

_GRAPH = None
LAST_RESULT = None     # BassKernelResults of the most recent run (for test.py)

# split the final output group's accumulation into two 256-col halves so
# half A's evac+store hide under half B's matmuls
SPLIT7 = False

# weight-chunk SBUF ring slots: deep enough that the sync sequencer's
# per-chunk descriptor generation (0.6-3.3us each, run-to-run variable)
# starts early enough for phase-B chunks to land before the PE reaches them.
CW_BUFS = 6


def _build_graph_raw():
    """Raw bacc build: manual per-engine streams + semaphores.  Saves the
    Tile exit drain + double all-engine barrier (~9us) and waits only once
    per weight chunk on the PE instead of per matmul."""
    import concourse.bass as bass
    from concourse import bacc, mybir

    nc = bacc.Bacc("TRN2", target_bir_lowering=False, debug=False,
                   num_devices=NCORES, monotonic_sem_count=0)
    f32 = mybir.dt.float32
    bf16 = mybir.dt.bfloat16

    xT = nc.dram_tensor("xT", [I, BS], bf16, kind="ExternalInput").ap()
    cw = nc.dram_tensor("cw", [128, OC * NJ * 128], bf16,
                        kind="ExternalInput").ap()
    bias = nc.dram_tensor("bias", [128, OC], f32, kind="ExternalInput").ap()
    yT = nc.dram_tensor("yT", [O, BS], f32, kind="ExternalOutput").ap()

    max_chunk = max(sz for _, sz in CHUNKS)
    xin = [nc.alloc_sbuf_tensor(f"xin{i}", [128, BS], bf16).ap()
           for i in range(IC)]
    planes = [nc.alloc_sbuf_tensor(f"pl{j}", [128, BS], bf16).ap()
              for j in range(NJ)]
    cwbuf = [nc.alloc_sbuf_tensor(f"cwb{i}", [128, max_chunk * 128],
                                  bf16).ap()
             for i in range(CW_BUFS)]
    # never written: garbage operand for PE p-state warm-up matmuls
    warm = nc.alloc_sbuf_tensor("warm", [128, 256], bf16).ap()
    warm2 = nc.alloc_sbuf_tensor("warm2", [128, BS], bf16).ap()
    bias_t = nc.alloc_sbuf_tensor("biasb", [128, OC], f32).ap()
    ot = [nc.alloc_sbuf_tensor(f"ot{i}", [128, BS], f32).ap()
          for i in range(2)]
    ps = [nc.alloc_psum_tensor(f"ps{i}", [128, BS], f32).ap()
          for i in range(OC)]
    HB = BS // 2
    ps7a, ps7b = ps[OC - 1][:, 0:HB], ps[OC - 1][:, HB:BS]

    from contextlib import ExitStack
    with ExitStack() as stack:
        # gpsimd issues only early DMAs whose completions are consumed mid-
        # kernel, so its expensive end-of-block dge_drain can be skipped
        block = stack.enter_context(nc.Block(no_gpsimd_drain=True))
        # DMA completion increments land as 16 per-slice +1s, and slices of
        # different in-flight DMAs interleave -- so a semaphore may only be
        # waited at "all DMAs issued on it so far" thresholds.  The weight
        # stream round-robins CW_BUFS semaphores (slot ring ensures only one
        # in-flight DMA per sem); x tiles get one sem each; output slots two.
        cw_dma = [stack.enter_context(nc.semaphore(f"cw_dma{r}"))
                  for r in range(CW_BUFS)]
        # SWDGE and HWDGE may not share a sem: the gpsimd-issued leading
        # chunks get dedicated sems even though they share the buffer ring
        cwg = [stack.enter_context(nc.semaphore(f"cwg{r}"))
               for r in range(N_GP)]
        # xin0 gates the first tanh and rides the sync queue ahead of the
        # weight stream; xins 1..7 go on gpsimd SWDGE with per-tile sems
        # (SWDGE and HWDGE DMAs may not mix on a sem)
        xin0_dma = stack.enter_context(nc.semaphore("xin0_dma"))
        xr_dma = [stack.enter_context(nc.semaphore(f"xr_dma{i}"))
                  for i in range(IC - 1)]
        bias_dma = stack.enter_context(nc.semaphore("bias_dma"))
        bo_dma = stack.enter_context(nc.semaphore("bo_dma"))
        out_dma = [stack.enter_context(nc.semaphore(f"out_dma{r}"))
                   for r in range(2)]
        act_pl = stack.enter_context(nc.semaphore("act_pl"))
        dve_pl = stack.enter_context(nc.semaphore("dve_pl"))
        pe_ch = stack.enter_context(nc.semaphore("pe_ch"))
        pe_half = stack.enter_context(nc.semaphore("pe_half"))
        act_ev = stack.enter_context(nc.semaphore("act_ev"))
        dve_ev = stack.enter_context(nc.semaphore("dve_ev"))

        @block.sync
        def _(eng: bass.BassEngine):
            # xin0 first: it gates the whole plane pipeline
            eng.dma_start(out=xin[0][:], in_=xT[0:128, :]
                          ).then_inc(xin0_dma, 16)
            for ci, (s0, size) in enumerate(CHUNKS):
                if ci < N_GP:
                    continue               # leading chunks ride gpsimd
                if ci >= CW_BUFS:
                    eng.wait_ge(pe_ch, ci - CW_BUFS + 1)
                eng.dma_start(
                    out=cwbuf[ci % CW_BUFS][:, :size * 128],
                    in_=cw[:, s0 * 128:(s0 + size) * 128],
                ).then_inc(cw_dma[ci % CW_BUFS], 16)
            # output stores: the evac->store handoff runs here so the DMA
            # issue cost (~0.6us each) never serializes with the next evac
            # on the Scalar queue
            for oc in range(OC - 1):
                eng.wait_ge(act_ev, oc + 1)
                eng.dma_start(
                    out=yT[oc * 128:(oc + 1) * 128, :],
                    in_=ot[oc % 2][:]
                ).then_inc(out_dma[oc % 2], 16)
            # last group: bias is already in PSUM (K=1 bias matmul), the
            # two halves are copied out by Scalar and Vector in parallel;
            # half A stores from here, half B from the Vector queue
            o0 = (OC - 1) * 128
            eng.wait_ge(act_ev, OC)
            eng.dma_start(out=yT[o0:o0 + 128, 0:HB], in_=ot[1][:, 0:HB]
                          ).then_inc(out_dma[1], 16)

        @block.gpsimd
        def _(eng: bass.BassEngine):
            # leading weight chunks + x tiles 1..7 + bias on the otherwise-
            # idle SWDGE queue, in consumption order
            for ci in range(N_GP):
                s0, size = CHUNKS[ci]
                eng.dma_start(
                    out=cwbuf[ci % CW_BUFS][:, :size * 128],
                    in_=cw[:, s0 * 128:(s0 + size) * 128],
                ).then_inc(cwg[ci], 16)
            for i in range(1, IC):
                eng.dma_start(out=xin[i][:], in_=xT[i * 128:(i + 1) * 128, :]
                              ).then_inc(xr_dma[i - 1], 16)
            eng.dma_start(out=bias_t[:], in_=bias[:]).then_inc(bias_dma, 16)

        @block.scalar
        def _(eng: bass.BassEngine):
            eng.wait_ge(xin0_dma, 16)
            eng.activation(planes[0][:], xin[0][:],
                           mybir.ActivationFunctionType.Tanh
                           ).then_inc(act_pl, 1)
            for i in range(1, IC):
                eng.wait_ge(xr_dma[i - 1], 16)
                eng.activation(planes[i * NK][:], xin[i][:],
                               mybir.ActivationFunctionType.Tanh
                               ).then_inc(act_pl, 1)
            eng.wait_ge(bias_dma, 16)
            for oc in range(OC - 1):
                eng.wait_ge(pe_ch, GROUP_END_CHUNK[oc] + 1)
                if oc >= 2:
                    eng.wait_ge(out_dma[oc % 2], 16 * (oc // 2))
                eng.activation(ot[oc % 2][:], ps[oc][:],
                               mybir.ActivationFunctionType.Identity,
                               bias=bias_t[:, oc:oc + 1]
                               ).then_inc(act_ev, 1)
            # last group, half A: pure copy (bias already accumulated in
            # PSUM); half B is copied by the Vector engine concurrently.
            # No final out-DMA waits -- the runtime drains the queues.
            eng.wait_ge(pe_ch, len(CHUNKS))
            eng.wait_ge(out_dma[1], 16 * ((OC - 1) // 2))
            eng.activation(ot[1][:, 0:HB], ps[OC - 1][:, 0:HB],
                           mybir.ActivationFunctionType.Identity,
                           bias=bias_t[:, OC - 1:OC]).then_inc(act_ev, 1)
            eng.activation(ot[1][:, HB:BS], ps[OC - 1][:, HB:BS],
                           mybir.ActivationFunctionType.Identity,
                           bias=bias_t[:, OC - 1:OC]).then_inc(dve_ev, 1)
            # store half B from here while Sync issues half A
            o0 = (OC - 1) * 128
            eng.wait_ge(dve_ev, 1)
            eng.dma_start(out=yT[o0:o0 + 128, HB:BS],
                          in_=ot[1][:, HB:BS]).then_inc(out_dma[1], 16)

        @block.vector
        def _(eng: bass.BassEngine):
            for i in range(IC):
                eng.wait_ge(act_pl, i + 1)
                for k1 in range(1, NK):
                    if k1 >= 2:
                        # same-engine RAW still needs a sem wait (deep
                        # pipeline, no interlock)
                        eng.wait_ge(dve_pl, i * (NK - 1) + k1 - 1)
                    eng.tensor_mul(planes[i * NK + k1][:],
                                   planes[i * NK + k1 - 1][:],
                                   planes[i * NK][:]
                                   ).then_inc(dve_pl, 1)


        @block.tensor
        def _(eng: bass.BassEngine):
            # p-state warm-up on garbage inputs: no waits, runs while the
            # first x tile + weight chunk DMAs are in flight, so the real
            # stream starts at (or near) full clock
            for _ in range(8):
                eng.matmul(ps[0][:], warm2[:, 0:128], warm2[:],
                           start=True, stop=True)
            doneA = [0] * OC
            doneB = 0
            seen_act = seen_dve = 0
            sem_uses = [0] * CW_BUFS   # HWDGE waits per ring sem
            for ci, (s0, size) in enumerate(CHUNKS):
                # attach all of the chunk's waits to its first matmul --
                # the move_matmul_waits_to_ldweights compile pass hoists
                # them onto the LDWEIGHTS, keeping the PE's 64-deep
                # reorder window free to pull later weight loads ahead
                js = [SEQ[s][1] for s in range(s0, s0 + size)]
                need_act = max((j // NK + 1 for j in js if j % NK == 0),
                               default=0)
                need_dve = max((j // NK * (NK - 1) + j % NK
                                for j in js if j % NK != 0), default=0)
                if need_act > seen_act:
                    eng.wait_ge(act_pl, need_act)
                    seen_act = need_act
                if need_dve > seen_dve:
                    eng.wait_ge(dve_pl, need_dve)
                    seen_dve = need_dve
                last7 = (ci == len(CHUNKS) - 1) and SPLIT7   # two sweeps
                for t in range(size):
                    oc, j = SEQ[s0 + t]
                    cwap = cwbuf[ci % CW_BUFS][:, t * 128:(t + 1) * 128]
                    if oc < OC - 1 or not SPLIT7:
                        mm = eng.matmul(ps[oc][:], cwap, planes[j][:],
                                        start=(doneA[oc] == 0),
                                        stop=(doneA[oc] == NJ - 1))
                        doneA[oc] += 1
                    else:
                        mm = eng.matmul(ps7a, cwap, planes[j][:, 0:HB],
                                        start=(doneA[oc] == 0),
                                        stop=(doneA[oc] == NJ - 1))
                        if doneA[oc] == NJ - 1:
                            mm.then_inc(pe_half, 1)
                        doneA[oc] += 1
                        if not last7:
                            # phase A: same stationary tile, second half
                            eng.matmul(ps7b, cwap, planes[j][:, HB:BS],
                                       start=(doneB == 0), stop=False)
                            doneB += 1
                    if t == 0:
                        if ci < N_GP:
                            mm._wait_ge(cwg[ci], 16)
                        else:
                            sem_uses[ci % CW_BUFS] += 1
                            mm._wait_ge(cw_dma[ci % CW_BUFS],
                                        16 * sem_uses[ci % CW_BUFS])
                    if t == size - 1 and not last7:
                        mm.then_inc(pe_ch, 1)
                if last7:
                    # second sweep: half B of the final group (re-loads the
                    # 42 stationary tiles; LDWEIGHTS hides under the 256-col
                    # matmuls)
                    for t in range(size):
                        _, j = SEQ[s0 + t]
                        mm = eng.matmul(
                            ps7b,
                            cwbuf[ci % CW_BUFS][:, t * 128:(t + 1) * 128],
                            planes[j][:, HB:BS],
                            start=(doneB == 0), stop=(doneB == NJ - 1))
                        doneB += 1
                        if t == size - 1:
                            mm.then_inc(pe_ch, 1)
            assert all(d == NJ for d in doneA)
            assert doneB == (NJ if SPLIT7 else 0)

    nc.compile()
    return nc


def _get_graph():
    global _GRAPH
    if _GRAPH is None:
        _GRAPH = _build_graph_raw()
    return _GRAPH


def _host_prep(a, q, coeffs):
    """Fold the polynomial basis change into the weights (float64 on host)."""
    # c[d, k]: P_d(t) = sum_k c[d, k] * t^k, from the three-term recurrence
    c = np.zeros((D1, D1), np.float64)
    c[0, 0] = 1.0
    if D1 > 1:
        c[1, 1] = 1.0
        c[1, 0] = -a
    for n in range(2, D1):
        c[n, 1:] += c[n - 1, :-1]
        c[n, :] -= (a + q ** n) * c[n - 1, :]
        c[n, :] -= a * q ** (n - 1) * c[n - 2, :]

    Cf = (coeffs.reshape(-1, D1).astype(np.float64) @ c).reshape(I, O, D1)
    bias = Cf[:, :, 0].sum(axis=0).astype(np.float32)                # [O]
    Ck = Cf[:, :, 1:].astype(np.float32).astype(ml_dtypes.bfloat16)  # [I,O,NK]

    # stationary tile for (oc, j=ic*NK+k1): [128 i-part, 128 o-col] slice
    t = Ck.reshape(IC, 128, OC, 128, NK)            # [ic, p, oc, ol, k1]
    X = np.ascontiguousarray(t.transpose(2, 0, 4, 1, 3)) \
          .reshape(OC, NJ, 128, 128)                # [oc, j, p, ol]
    oc_idx = np.array([oc for oc, _ in SEQ])
    j_idx = np.array([j for _, j in SEQ])
    seq_tiles = X[oc_idx, j_idx]                    # [448, p, ol]
    cw_dev = np.ascontiguousarray(
        seq_tiles.transpose(1, 0, 2)).reshape(128, OC * NJ * 128)
    bias_dev = np.ascontiguousarray(bias.reshape(OC, 128).T)  # [128, OC]
    return cw_dev, bias_dev


def _ensure_axon_hooks_importable():
    """run_bass_kernel_spmd imports antenv.axon_hooks when BASS_TRACE is
    set; some images lack that module.  Register a no-op fallback so a
    trace request degrades to a warning instead of an ImportError."""
    import sys
    import types
    if "antenv.axon_hooks" in sys.modules:
        return
    try:
        import antenv.axon_hooks  # noqa: F401
    except ImportError:
        mod = types.ModuleType("antenv.axon_hooks")
        state = {"hook": None}
        mod.set_axon_ntff_profile_hook = \
            lambda h: state.__setitem__("hook", h)
        mod.get_axon_ntff_profile_hook = lambda: state["hook"]
        sys.modules["antenv.axon_hooks"] = mod
        try:
            import antenv
            antenv.axon_hooks = mod
        except ImportError:
            pass


def kernel(x, a, q, coeffs):
    global LAST_RESULT
    _ensure_axon_hooks_importable()
    from concourse.bass_utils import run_bass_kernel_spmd

    x = np.ascontiguousarray(np.asarray(x, dtype=np.float32))
    coeffs = np.ascontiguousarray(np.asarray(coeffs, dtype=np.float32))
    a_val = float(np.asarray(a).reshape(-1)[0])
    q_val = float(np.asarray(q).reshape(-1)[0])

    cw_dev, bias_dev = _host_prep(a_val, q_val, coeffs)
    xs = x.reshape(NCORES, BS, I).transpose(0, 2, 1)  # [core, I, BS]
    xs = xs.astype(ml_dtypes.bfloat16)

    in_maps = [{
        "xT": np.ascontiguousarray(xs[c]),
        "cw": cw_dev,
        "bias": bias_dev,
    } for c in range(NCORES)]

    nc = _get_graph()
    res = run_bass_kernel_spmd(nc, in_maps, core_ids=list(range(NCORES)))
    LAST_RESULT = res

    shards = [np.asarray(res.results[c]["yT"]).T for c in range(NCORES)]
    return np.ascontiguousarray(np.concatenate(shards, axis=0),
                                dtype=np.float32)


if __name__ == "__main__":
    rng = np.random.default_rng(0)
    inputs = {
        "x": rng.standard_normal((B, I), dtype=np.float32),
        "a": np.zeros((1,), np.float32),
        "q": np.ones((1,), np.float32),
        "coeffs": rng.standard_normal((I, O, D1), dtype=np.float32)
        / (I * D1),
    }
    y = kernel(**inputs)
    print("out", y.shape, y.dtype, float(np.abs(y).mean()))
